# revision 10
# baseline (speedup 1.0000x reference)
"""KNN (B=4, N=8192, M=4096, d=3, k=16) on 8 Trainium2 cores.

Sharding: data-parallel over flattened (B*M)=16384 query rows -> 2048
rows/core; core c handles batch c//2 (refs not sharded; each core gets
its batch's full ref set).

v4 strategy (baseline was GPSIMD-elementwise-bound at 4.2ms):
  - PE (fp32, K=3) computes c2 = q.(2r) per [128,512] chunk (start=True,
    pure write - deterministic). Bit-exact vs jax einsum (verified).
  - Scalar engine (ACT) evicts c2 PSUM->SBUF (Copy) and computes
    S = fl(r2 + q2) via one big activation (Identity, bias=q2[p]).
  - GPSIMD computes T = fl(c2 - S) = -d2 in ONE [128,8192] tensor_sub
    (in-place on the c2 buffer). Exactly -round(S - c2): reference
    rounding. GPSIMD is otherwise idle; one op per tile amortizes its
    ~2.2us launch overhead.
  - DVE does pure top-16 selection on T (max8/max_index/match_replace/
    max8/max_index): largest T = smallest d2 = smallest dist, ties by
    lowest index (= lax.top_k on -dist; selection equivalence verified
    on this dataset including relu-clamp ties).
  - dist = sqrt(relu(-T)) on the 16 selected values only.

Engine busy per core: PE ~710us, DVE ~700us, ACT ~290us, GPSIMD ~250us.
"""

import numpy as np

_B, _N, _M, _D, _K = 4, 8192, 4096, 3, 16
_NCORES = 8
_QPC = (_B * _M) // _NCORES  # 2048 query rows per core
_QT = 128                    # queries per tile (partition dim)
_NT = _QPC // _QT            # 16 tiles per core
_CH = 512                    # matmul free-dim chunk (1 PSUM bank)
_NCH = _N // _CH             # 16 chunks

_nc_cache = None
_last_in_maps = None


def _build():
    import concourse.bacc as bacc
    import concourse.mybir as mybir
    from concourse import tile

    f32 = mybir.dt.float32
    u32 = mybir.dt.uint32
    AF = mybir.ActivationFunctionType

    nc = bacc.Bacc("TRN2", target_bir_lowering=False, debug=False)
    qs = nc.dram_tensor("qs", [3, _QPC], f32, kind="ExternalInput").ap()
    rm = nc.dram_tensor("rm", [3, _N], f32, kind="ExternalInput").ap()
    q2t = nc.dram_tensor("q2t", [_QT, _NT], f32, kind="ExternalInput").ap()
    r2b = nc.dram_tensor("r2b", [_QT, _N], f32, kind="ExternalInput").ap()
    dist = nc.dram_tensor("dist", [_QPC, _K], f32, kind="ExternalOutput").ap()
    idx = nc.dram_tensor("idx", [_QPC, _K], u32, kind="ExternalOutput").ap()

    with tile.TileContext(nc) as tc:
        with (
            tc.tile_pool(name="const", bufs=1) as cpool,
            tc.tile_pool(name="qp", bufs=2) as qpool,
            tc.tile_pool(name="tp", bufs=3) as tpool,
            tc.tile_pool(name="sp", bufs=5) as spool,
            tc.tile_pool(name="ps", bufs=8, space="PSUM") as ppool,
            tc.tile_pool(name="outs", bufs=3) as opool,
        ):
            rm_t = cpool.tile([3, _N], f32)
            q2t_t = cpool.tile([_QT, _NT], f32)
            r2b_t = cpool.tile([_QT, _N], f32)

            # PE HAM warmup: dummy matmuls on whatever is in r2b's SBUF
            # before its DMA lands (WAR dep orders the DMA after us).
            for _ in range(3):
                pw = ppool.tile([_QT, _CH], f32, tag="ps")
                nc.tensor.matmul(pw[:], r2b_t[0:3, 0:_QT],
                                 r2b_t[0:3, 0:_CH], start=True, stop=True)

            nc.sync.dma_start(rm_t[:], rm[:])
            nc.sync.dma_start(q2t_t[:], q2t[:])
            for c in range(_NCH):
                sl = slice(c * _CH, (c + 1) * _CH)
                nc.sync.dma_start(r2b_t[:, sl], r2b[:, sl])

            _QW = _N // 4   # 2048-wide quarter for S / gpsimd subtract
            for t in range(_NT):
                T = tpool.tile([_QT, _N], f32, tag="T")
                qs_tt = qpool.tile([3, _QT], f32, tag="q")
                nc.sync.dma_start(qs_tt[:], qs[:, t * _QT:(t + 1) * _QT])
                Sq = [None] * 4
                for c in range(_NCH):
                    sl = slice(c * _CH, (c + 1) * _CH)
                    if c % 4 == 0:
                        i = c // 4
                        qsl = slice(i * _QW, (i + 1) * _QW)
                        Sq[i] = spool.tile([_QT, _QW], f32, tag="S", name=f"Sq{t}_{i}")
                        # S = fl(r2 + q2), one rounding (reference order)
                        nc.scalar.activation(
                            Sq[i][:], r2b_t[:, qsl], AF.Identity,
                            bias=q2t_t[:, t:t + 1], scale=1.0)
                    ps = ppool.tile([_QT, _CH], f32, tag="ps")
                    nc.tensor.matmul(
                        ps[:],
                        qs_tt[:],
                        rm_t[:, sl],
                        start=True,
                        stop=True,
                    )
                    nc.scalar.activation(T[:, sl], ps[:], AF.Copy)
                    if c % 4 == 3:
                        # T quarter = fl(c2 - S) = -d2 (reference rounding)
                        i = c // 4
                        qsl = slice(i * _QW, (i + 1) * _QW)
                        nc.gpsimd.tensor_sub(
                            T[:, qsl], T[:, qsl], Sq[i][:])

                v = opool.tile([_QT, _K], f32, tag="v")
                ix = opool.tile([_QT, _K], u32, tag="ix")
                nc.vector.max(v[:, 0:8], T[:])
                nc.vector.max_index(ix[:, 0:8], v[:, 0:8], T[:])
                nc.vector.match_replace(T[:], v[:, 0:8], T[:], -1.0e30)
                nc.vector.max(v[:, 8:16], T[:])
                nc.vector.max_index(ix[:, 8:16], v[:, 8:16], T[:])

                # dist = sqrt(relu(-T)) on the 16 selected values only
                d = opool.tile([_QT, _K], f32, tag="d")
                nc.scalar.activation(d[:], v[:], AF.Relu, scale=-1.0)
                nc.scalar.activation(d[:], d[:], AF.Sqrt)
                nc.sync.dma_start(dist[t * _QT:(t + 1) * _QT, :], d[:])
                nc.sync.dma_start(idx[t * _QT:(t + 1) * _QT, :], ix[:])
    nc.compile()
    return nc


def kernel(ref: np.ndarray, query: np.ndarray, k) -> tuple:
    global _nc_cache, _last_in_maps
    from concourse.bass_utils import run_bass_kernel_spmd

    assert int(k) == _K
    ref = np.asarray(ref, dtype=np.float32)
    query = np.asarray(query, dtype=np.float32)

    fq = query.reshape(_B * _M, _D)
    in_maps = []
    for c in range(_NCORES):
        q = fq[c * _QPC:(c + 1) * _QPC]              # [2048, 3]
        r = ref[(c * _QPC) // _M]                    # [8192, 3]
        q2 = np.sum(q * q, axis=1, dtype=np.float32)
        r2 = np.sum(r * r, axis=1, dtype=np.float32)
        in_maps.append({
            "qs": np.ascontiguousarray(q.T),
            "rm": np.ascontiguousarray(2.0 * r.T),
            "q2t": np.ascontiguousarray(q2.reshape(_NT, _QT).T),
            "r2b": np.ascontiguousarray(np.broadcast_to(r2, (_QT, _N))),
        })

    _last_in_maps = in_maps
    if _nc_cache is None:
        _nc_cache = _build()
    res = run_bass_kernel_spmd(_nc_cache, in_maps, list(range(_NCORES)))

    D = np.empty((_B * _M, _K), np.float32)
    I = np.empty((_B * _M, _K), np.int32)
    for c in range(_NCORES):
        D[c * _QPC:(c + 1) * _QPC] = res.results[c]["dist"]
        I[c * _QPC:(c + 1) * _QPC] = res.results[c]["idx"].astype(np.int32)
    return D.reshape(_B, _M, _K), I.reshape(_B, _M, _K)


# revision 12
# speedup vs baseline: 1.1869x; 1.1869x over previous
"""KNN (B=4, N=8192, M=4096, d=3, k=16) on 8 Trainium2 cores.

Sharding: data-parallel over flattened (B*M)=16384 query rows -> 2048
rows/core; core c handles batch c//2 (refs not sharded; each core gets
its batch's full ref set).

v4 strategy (baseline was GPSIMD-elementwise-bound at 4.2ms):
  - PE (fp32, K=3) computes c2 = q.(2r) per [128,512] chunk (start=True,
    pure write - deterministic). Bit-exact vs jax einsum (verified).
  - Scalar engine (ACT) evicts c2 PSUM->SBUF (Copy) and computes
    S = fl(r2 + q2) via one big activation (Identity, bias=q2[p]).
  - GPSIMD computes T = fl(c2 - S) = -d2 in ONE [128,8192] tensor_sub
    (in-place on the c2 buffer). Exactly -round(S - c2): reference
    rounding. GPSIMD is otherwise idle; one op per tile amortizes its
    ~2.2us launch overhead.
  - DVE does pure top-16 selection on T (max8/max_index/match_replace/
    max8/max_index): largest T = smallest d2 = smallest dist, ties by
    lowest index (= lax.top_k on -dist; selection equivalence verified
    on this dataset including relu-clamp ties).
  - dist = sqrt(relu(-T)) on the 16 selected values only.

Engine busy per core: PE ~710us, DVE ~700us, ACT ~290us, GPSIMD ~250us.
"""

import numpy as np

_B, _N, _M, _D, _K = 4, 8192, 4096, 3, 16
_NCORES = 8
_QPC = (_B * _M) // _NCORES  # 2048 query rows per core
_QT = 128                    # queries per tile (partition dim)
_NT = _QPC // _QT            # 16 tiles per core
_CH = 512                    # matmul free-dim chunk (1 PSUM bank)
_NCH = _N // _CH             # 16 chunks

_nc_cache = None
_last_in_maps = None


def _build():
    import concourse.bacc as bacc
    import concourse.mybir as mybir
    from concourse import tile

    f32 = mybir.dt.float32
    u32 = mybir.dt.uint32
    AF = mybir.ActivationFunctionType

    nc = bacc.Bacc("TRN2", target_bir_lowering=False, debug=False)
    qs = nc.dram_tensor("qs", [3, _QPC], f32, kind="ExternalInput").ap()
    rm = nc.dram_tensor("rm", [3, _N], f32, kind="ExternalInput").ap()
    q2t = nc.dram_tensor("q2t", [_QT, _NT], f32, kind="ExternalInput").ap()
    r2b = nc.dram_tensor("r2b", [_QT, _N], f32, kind="ExternalInput").ap()
    dist = nc.dram_tensor("dist", [_QPC, _K], f32, kind="ExternalOutput").ap()
    idx = nc.dram_tensor("idx", [_QPC, _K], u32, kind="ExternalOutput").ap()

    with tile.TileContext(nc) as tc:
        with (
            tc.tile_pool(name="const", bufs=1) as cpool,
            tc.tile_pool(name="qp", bufs=2) as qpool,
            tc.tile_pool(name="tp", bufs=3) as tpool,
            tc.tile_pool(name="sp", bufs=5) as spool,
            tc.tile_pool(name="ps", bufs=8, space="PSUM") as ppool,
            tc.tile_pool(name="outs", bufs=3) as opool,
        ):
            rm_t = cpool.tile([3, _N], f32)
            q2t_t = cpool.tile([_QT, _NT], f32)
            r2b_t = cpool.tile([_QT, _N], f32)

            # PE HAM warmup: dummy matmuls on whatever is in r2b's SBUF
            # before its DMA lands (WAR dep orders the DMA after us).
            for _ in range(3):
                pw = ppool.tile([_QT, _CH], f32, tag="ps")
                nc.tensor.matmul(pw[:], r2b_t[0:3, 0:_QT],
                                 r2b_t[0:3, 0:_CH], start=True, stop=True)

            nc.sync.dma_start(rm_t[:], rm[:])
            nc.sync.dma_start(q2t_t[:], q2t[:])
            for c in range(_NCH):
                sl = slice(c * _CH, (c + 1) * _CH)
                nc.sync.dma_start(r2b_t[:, sl], r2b[:, sl])

            _QW = _N // 4   # 2048-wide quarter for S / gpsimd subtract
            for t in range(_NT):
                T = tpool.tile([_QT, _N], f32, tag="T")
                qs_tt = qpool.tile([3, _QT], f32, tag="q")
                nc.sync.dma_start(qs_tt[:], qs[:, t * _QT:(t + 1) * _QT])
                Sq = [None] * 4
                for c in range(_NCH):
                    sl = slice(c * _CH, (c + 1) * _CH)
                    if c % 4 == 0:
                        i = c // 4
                        qsl = slice(i * _QW, (i + 1) * _QW)
                        Sq[i] = spool.tile([_QT, _QW], f32, tag="S", name=f"Sq{t}_{i}")
                        # Sneg = fl(-r2 - q2) = -fl(r2 + q2), one rounding
                        nc.scalar.activation(
                            Sq[i][:], r2b_t[:, qsl], AF.Identity,
                            bias=q2t_t[:, t:t + 1], scale=-1.0)
                    ps = ppool.tile([_QT, _CH], f32, tag="ps")
                    nc.tensor.matmul(
                        ps[:],
                        qs_tt[:],
                        rm_t[:, sl],
                        start=True,
                        stop=True,
                    )
                    nc.scalar.activation(T[:, sl], ps[:], AF.Copy)
                    if c % 4 == 3:
                        # T quarter += -S via DMA accumulate:
                        # fl(c2 - S) = -d2 (reference rounding)
                        i = c // 4
                        qsl = slice(i * _QW, (i + 1) * _QW)
                        nc.gpsimd.dma_start(
                            T[:, qsl], Sq[i][:],
                            accum_op=mybir.AluOpType.add)

                v = opool.tile([_QT, _K], f32, tag="v")
                ix = opool.tile([_QT, _K], u32, tag="ix")
                nc.vector.max(v[:, 0:8], T[:])
                nc.vector.max_index(ix[:, 0:8], v[:, 0:8], T[:])
                nc.vector.match_replace(T[:], v[:, 0:8], T[:], -1.0e30)
                nc.vector.max(v[:, 8:16], T[:])
                nc.vector.max_index(ix[:, 8:16], v[:, 8:16], T[:])

                # dist = sqrt(relu(-T)) on the 16 selected values only
                d = opool.tile([_QT, _K], f32, tag="d")
                nc.scalar.activation(d[:], v[:], AF.Relu, scale=-1.0)
                nc.scalar.activation(d[:], d[:], AF.Sqrt)
                nc.sync.dma_start(dist[t * _QT:(t + 1) * _QT, :], d[:])
                nc.sync.dma_start(idx[t * _QT:(t + 1) * _QT, :], ix[:])
    nc.compile()
    return nc


def kernel(ref: np.ndarray, query: np.ndarray, k) -> tuple:
    global _nc_cache, _last_in_maps
    from concourse.bass_utils import run_bass_kernel_spmd

    assert int(k) == _K
    ref = np.asarray(ref, dtype=np.float32)
    query = np.asarray(query, dtype=np.float32)

    fq = query.reshape(_B * _M, _D)
    in_maps = []
    for c in range(_NCORES):
        q = fq[c * _QPC:(c + 1) * _QPC]              # [2048, 3]
        r = ref[(c * _QPC) // _M]                    # [8192, 3]
        q2 = np.sum(q * q, axis=1, dtype=np.float32)
        r2 = np.sum(r * r, axis=1, dtype=np.float32)
        in_maps.append({
            "qs": np.ascontiguousarray(q.T),
            "rm": np.ascontiguousarray(2.0 * r.T),
            "q2t": np.ascontiguousarray((-q2).reshape(_NT, _QT).T),
            "r2b": np.ascontiguousarray(np.broadcast_to(r2, (_QT, _N))),
        })

    _last_in_maps = in_maps
    if _nc_cache is None:
        _nc_cache = _build()
    res = run_bass_kernel_spmd(_nc_cache, in_maps, list(range(_NCORES)))

    D = np.empty((_B * _M, _K), np.float32)
    I = np.empty((_B * _M, _K), np.int32)
    for c in range(_NCORES):
        D[c * _QPC:(c + 1) * _QPC] = res.results[c]["dist"]
        I[c * _QPC:(c + 1) * _QPC] = res.results[c]["idx"].astype(np.int32)
    return D.reshape(_B, _M, _K), I.reshape(_B, _M, _K)


# revision 14
# speedup vs baseline: 1.1941x; 1.0061x over previous
"""KNN (B=4, N=8192, M=4096, d=3, k=16) on 8 Trainium2 cores.

Sharding: data-parallel over flattened (B*M)=16384 query rows -> 2048
rows/core; core c handles batch c//2 (refs not sharded; each core gets
its batch's full ref set).

v6 strategy (baseline was GPSIMD-elementwise-bound at 4.2ms; measured
771us here, bit-exact):
  - PE (fp32, K=3) computes c2 = q.(2r) per [128,512] chunk (start=True,
    pure write - deterministic). Bit-exact vs jax einsum (verified).
  - Scalar engine (ACT) evicts c2 PSUM->SBUF (Copy) into T and computes
    Sneg = fl(-r2 - q2) = -fl(r2+q2) per 2048-wide quarter
    (Identity, scale=-1, bias=-q2[p]).
  - Accumulate-DMA (software DGE on the gpsimd queue; data moves on DMA
    engines, so no SBUF-port contention with DVE/PE) does
    T += Sneg per quarter: T = fl(c2 - S) = -d2 with reference-identical
    rounding (-round(S - c2)). GPSIMD compute ops and DVE elementwise
    passes both proved slower here (engine contention / extra passes).
  - DVE does pure top-16 selection on T (max8/max_index/match_replace/
    max8/max_index): largest T = smallest d2 = smallest dist, ties by
    lowest index (= lax.top_k on -dist; selection equivalence incl.
    relu-clamp ties verified on this dataset).
  - dist = sqrt(relu(-T)) on the 16 selected values only (ACT).

Engine busy per core: DVE ~700us (bottleneck), PE ~630us, ACT ~305us,
DMA ~360us total across 16 engines. Runtime = DVE busy + ~50us pipeline
fill + ~25us tail.
"""

import numpy as np

_B, _N, _M, _D, _K = 4, 8192, 4096, 3, 16
_NCORES = 8
_QPC = (_B * _M) // _NCORES  # 2048 query rows per core
_QT = 128                    # queries per tile (partition dim)
_NT = _QPC // _QT            # 16 tiles per core
_CH = 512                    # matmul free-dim chunk (1 PSUM bank)
_NCH = _N // _CH             # 16 chunks

_nc_cache = None
_last_in_maps = None


def _build():
    import concourse.bacc as bacc
    import concourse.mybir as mybir
    from concourse import tile

    f32 = mybir.dt.float32
    u32 = mybir.dt.uint32
    AF = mybir.ActivationFunctionType

    nc = bacc.Bacc("TRN2", target_bir_lowering=False, debug=False)
    qs = nc.dram_tensor("qs", [3, _QPC], f32, kind="ExternalInput").ap()
    rm = nc.dram_tensor("rm", [3, _N], f32, kind="ExternalInput").ap()
    q2t = nc.dram_tensor("q2t", [_QT, _NT], f32, kind="ExternalInput").ap()
    r2b = nc.dram_tensor("r2b", [_QT, _N], f32, kind="ExternalInput").ap()
    dist = nc.dram_tensor("dist", [_QPC, _K], f32, kind="ExternalOutput").ap()
    idx = nc.dram_tensor("idx", [_QPC, _K], u32, kind="ExternalOutput").ap()

    with tile.TileContext(nc) as tc:
        with (
            tc.tile_pool(name="const", bufs=1) as cpool,
            tc.tile_pool(name="qp", bufs=2) as qpool,
            tc.tile_pool(name="tp", bufs=3) as tpool,
            tc.tile_pool(name="sp", bufs=5) as spool,
            tc.tile_pool(name="ps", bufs=8, space="PSUM") as ppool,
            tc.tile_pool(name="outs", bufs=3) as opool,
        ):
            rm_t = cpool.tile([3, _N], f32)
            q2t_t = cpool.tile([_QT, _NT], f32)
            r2b_t = cpool.tile([_QT, _N], f32)

            # PE HAM warmup: dummy matmuls on whatever is in r2b's SBUF
            # before its DMA lands (WAR dep orders the DMA after us).
            for _ in range(3):
                pw = ppool.tile([_QT, _CH], f32, tag="ps")
                nc.tensor.matmul(pw[:], r2b_t[0:3, 0:_QT],
                                 r2b_t[0:3, 0:_CH], start=True, stop=True)

            nc.sync.dma_start(rm_t[:], rm[:])
            nc.sync.dma_start(q2t_t[:], q2t[:])
            for c in range(_NCH):
                sl = slice(c * _CH, (c + 1) * _CH)
                nc.sync.dma_start(r2b_t[:, sl], r2b[:, sl])

            _QW = _N // 4   # 2048-wide quarter for S / gpsimd subtract
            for t in range(_NT):
                T = tpool.tile([_QT, _N], f32, tag="T")
                qs_tt = qpool.tile([3, _QT], f32, tag="q")
                nc.sync.dma_start(qs_tt[:], qs[:, t * _QT:(t + 1) * _QT])
                Sq = [None] * 4
                for c in range(_NCH):
                    sl = slice(c * _CH, (c + 1) * _CH)
                    if c % 4 == 0:
                        i = c // 4
                        qsl = slice(i * _QW, (i + 1) * _QW)
                        Sq[i] = spool.tile([_QT, _QW], f32, tag="S", name=f"Sq{t}_{i}")
                        # Sneg = fl(-r2 - q2) = -fl(r2 + q2), one rounding
                        nc.scalar.activation(
                            Sq[i][:], r2b_t[:, qsl], AF.Identity,
                            bias=q2t_t[:, t:t + 1], scale=-1.0)
                    ps = ppool.tile([_QT, _CH], f32, tag="ps")
                    nc.tensor.matmul(
                        ps[:],
                        qs_tt[:],
                        rm_t[:, sl],
                        start=True,
                        stop=True,
                    )
                    nc.scalar.activation(T[:, sl], ps[:], AF.Copy)
                    # T chunk += -S via DMA accumulate:
                    # fl(c2 - S) = -d2 (reference rounding)
                    i = c // 4
                    ssl = slice((c % 4) * _CH, (c % 4 + 1) * _CH)
                    nc.gpsimd.dma_start(
                        T[:, sl], Sq[i][:, ssl],
                        accum_op=mybir.AluOpType.add)

                v = opool.tile([_QT, _K], f32, tag="v")
                ix = opool.tile([_QT, _K], u32, tag="ix")
                nc.vector.max(v[:, 0:8], T[:])
                nc.vector.max_index(ix[:, 0:8], v[:, 0:8], T[:])
                nc.vector.match_replace(T[:], v[:, 0:8], T[:], -1.0e30)
                nc.vector.max(v[:, 8:16], T[:])
                nc.vector.max_index(ix[:, 8:16], v[:, 8:16], T[:])

                # dist = sqrt(relu(-T)) on the 16 selected values only
                d = opool.tile([_QT, _K], f32, tag="d")
                nc.scalar.activation(d[:], v[:], AF.Relu, scale=-1.0)
                nc.scalar.activation(d[:], d[:], AF.Sqrt)
                nc.sync.dma_start(dist[t * _QT:(t + 1) * _QT, :], d[:])
                nc.sync.dma_start(idx[t * _QT:(t + 1) * _QT, :], ix[:])
    nc.compile()
    return nc


def kernel(ref: np.ndarray, query: np.ndarray, k) -> tuple:
    global _nc_cache, _last_in_maps
    from concourse.bass_utils import run_bass_kernel_spmd

    assert int(k) == _K
    ref = np.asarray(ref, dtype=np.float32)
    query = np.asarray(query, dtype=np.float32)

    fq = query.reshape(_B * _M, _D)
    in_maps = []
    for c in range(_NCORES):
        q = fq[c * _QPC:(c + 1) * _QPC]              # [2048, 3]
        r = ref[(c * _QPC) // _M]                    # [8192, 3]
        q2 = np.sum(q * q, axis=1, dtype=np.float32)
        r2 = np.sum(r * r, axis=1, dtype=np.float32)
        in_maps.append({
            "qs": np.ascontiguousarray(q.T),
            "rm": np.ascontiguousarray(2.0 * r.T),
            "q2t": np.ascontiguousarray((-q2).reshape(_NT, _QT).T),
            "r2b": np.ascontiguousarray(np.broadcast_to(r2, (_QT, _N))),
        })

    _last_in_maps = in_maps
    if _nc_cache is None:
        _nc_cache = _build()
    res = run_bass_kernel_spmd(_nc_cache, in_maps, list(range(_NCORES)))

    D = np.empty((_B * _M, _K), np.float32)
    I = np.empty((_B * _M, _K), np.int32)
    for c in range(_NCORES):
        D[c * _QPC:(c + 1) * _QPC] = res.results[c]["dist"]
        I[c * _QPC:(c + 1) * _QPC] = res.results[c]["idx"].astype(np.int32)
    return D.reshape(_B, _M, _K), I.reshape(_B, _M, _K)
